# revision 1
# baseline (speedup 1.0000x reference)
"""MiniGPT forward (single-head causal attention + vocab head) on 8 Trainium2
NeuronCores.

Sharding: core c = b*4 + j handles batch b and query block j (512 queries).
Each core receives its batch's token ids ROLLED left by 512*j so that its
query block always occupies rolled positions [0, 512) -- this keeps the SPMD
program identical across cores (all per-core differences live in the input
data). Keys/values cover the full (rolled) sequence; the causal mask for the
rolled layout is (t <= s) | (t >= 2048 - off), built on-chip from an
affine_select triangle plus a per-core wrap-column threshold.

The vocab head streams wo in 64 chunks of 500 columns. Matmuls run in
float32r (full-rate fp32 mode, ~1.5e-4 rel err); transposes stay exact fp32.
"""

import sys

sys.path.insert(0, "/opt/trn_rl_repo")

import numpy as np

import concourse.bass as bass
import concourse.bacc as bacc
import concourse.mybir as mybir
import concourse.tile as tile
from concourse.bass_utils import run_bass_kernel_spmd
from concourse.masks import make_identity

P = 128
S = 2048          # sequence / window
D = 1024          # model dim
V = 32000         # vocab
SB = 512          # query block per core
ST = S // P       # 16 sequence tiles
DT = D // P       # 8 model-dim tiles
SBT = SB // P     # 4 query tiles
NCH = 64
NW = V // NCH     # 500 vocab cols per head chunk

f32 = mybir.dt.float32
f32r = mybir.dt.float32r
i32 = mybir.dt.int32
AF = mybir.ActivationFunctionType
OP = mybir.AluOpType

NEG = -1.0e9


def _emit(nc):
    x = nc.declare_dram_parameter("x", [S], i32, isOutput=False)
    pos_t = nc.declare_dram_parameter("pos_t", [D, S], f32r, isOutput=False)
    cwrap = nc.declare_dram_parameter("cwrap", [P, 1], f32, isOutput=False)
    tok = nc.declare_dram_parameter("tok", [V, D], f32, isOutput=False)
    wq = nc.declare_dram_parameter("wq", [D, D], f32r, isOutput=False)
    wk = nc.declare_dram_parameter("wk", [D, D], f32r, isOutput=False)
    wv = nc.declare_dram_parameter("wv", [D, D], f32r, isOutput=False)
    bq = nc.declare_dram_parameter("bq", [D], f32, isOutput=False)
    bk = nc.declare_dram_parameter("bk", [D], f32, isOutput=False)
    bv = nc.declare_dram_parameter("bv", [D], f32r, isOutput=False)
    wo = nc.declare_dram_parameter("wo", [D, V], f32r, isOutput=False)
    bo = nc.declare_dram_parameter("bo", [V], f32r, isOutput=False)
    logits = nc.declare_dram_parameter("logits", [SB, V], f32, isOutput=True)

    v_buf = nc.dram_tensor("v_buf", [S, D], f32r)
    aT_buf = nc.dram_tensor("aT_buf", [S, SB], f32r)
    wo_r = wo[:].rearrange("(kt p) v -> p kt v", p=P)

    with tile.TileContext(nc, pool_alloc_mode="queue") as tc:
        _open = {}

        def popen(name, **kw):
            cm = tc.tile_pool(name=name, **kw)
            _open[name] = cm
            return cm.__enter__()

        def pclose(name):
            _open.pop(name).__exit__(None, None, None)

        misc = popen("misc", bufs=1)
        ident = misc.tile([P, P], f32)
        make_identity(nc, ident[:])
        ones_f = misc.tile([1, P], f32)
        nc.vector.memset(ones_f[:], 1.0)
        ones_r = misc.tile([1, P], f32r)
        nc.scalar.copy(ones_r[:], ones_f[:])

        # ---------------- phase A: gather + transpose -> hT ----------------
        # hT[d] starts as (rolled, transposed) pos_emb; transposed token
        # embedding blocks are accumulated into it.
        hTp = popen("hTp", bufs=1)
        hT = [hTp.tile([P, S], f32r, tag=f"hT{d}", name=f"hT{d}") for d in range(DT)]
        for d in range(DT):
            nc.sync.dma_start(hT[d][:], pos_t[d * P:(d + 1) * P, :])
        with (
            tc.tile_pool(name="ep", bufs=3) as ep,
            tc.tile_pool(name="psA", bufs=6, space="PSUM") as psA,
        ):
            for st in range(ST):
                idx = ep.tile([P, 1], i32, tag="idx", name="idx")
                nc.sync.dma_start(idx[:], x[st * P:(st + 1) * P, None])
                e = ep.tile([P, D], f32, tag="e", name="e")
                nc.gpsimd.indirect_dma_start(
                    out=e[:], out_offset=None, in_=tok[:],
                    in_offset=bass.IndirectOffsetOnAxis(ap=idx[:, :1], axis=0))
                for d in range(DT):
                    ps = psA.tile([P, P], f32, tag="tp", name="tp")
                    nc.tensor.transpose(ps[:], e[:, d * P:(d + 1) * P], ident[:])
                    nc.vector.tensor_tensor(
                        out=hT[d][:, st * P:(st + 1) * P],
                        in0=ps[:], in1=hT[d][:, st * P:(st + 1) * P], op=OP.add)

        # ------------- phases B/C/D: kT, qT, v (fp32r matmuls) -------------
        ktq = popen("ktq", bufs=1)
        kT = [ktq.tile([P, S], f32r, tag=f"kT{d}", name=f"kT{d}") for d in range(DT)]
        qT = [ktq.tile([P, SB], f32r, tag=f"qT{d}", name=f"qT{d}") for d in range(DT)]

        with (
            tc.tile_pool(name="wkp", bufs=1) as wkp,
            tc.tile_pool(name="psB", bufs=4, space="PSUM") as psB,
        ):
            bk_col = wkp.tile([P, DT], f32)
            nc.sync.dma_start(bk_col[:], bk[:].rearrange("(dt p) -> p dt", p=P))
            wk_t = []
            for kt in range(DT):
                w = wkp.tile([P, D], f32r, tag=f"wk{kt}", name=f"wk{kt}")
                nc.sync.dma_start(w[:], wk[kt * P:(kt + 1) * P, :])
                wk_t.append(w)
            for d in range(DT):
                for ch in range(S // 512):
                    ps = psB.tile([P, 512], f32, tag="mm", name="mm")
                    for kt in range(DT):
                        nc.tensor.matmul(
                            ps[:], wk_t[kt][:, d * P:(d + 1) * P],
                            hT[kt][:, ch * 512:(ch + 1) * 512],
                            start=(kt == 0), stop=(kt == DT - 1))
                    nc.scalar.activation(kT[d][:, ch * 512:(ch + 1) * 512],
                                         ps[:], AF.Identity,
                                         bias=bk_col[:, d:d + 1])

        with (
            tc.tile_pool(name="wqp", bufs=1) as wqp,
            tc.tile_pool(name="psC", bufs=4, space="PSUM") as psC,
        ):
            bq_col = wqp.tile([P, DT], f32)
            nc.sync.dma_start(bq_col[:], bq[:].rearrange("(dt p) -> p dt", p=P))
            wq_t = []
            for kt in range(DT):
                w = wqp.tile([P, D], f32r, tag=f"wq{kt}", name=f"wq{kt}")
                nc.sync.dma_start(w[:], wq[kt * P:(kt + 1) * P, :])
                wq_t.append(w)
            for d in range(DT):
                ps = psC.tile([P, 512], f32, tag="mm", name="mm")
                for kt in range(DT):
                    nc.tensor.matmul(
                        ps[:], wq_t[kt][:, d * P:(d + 1) * P],
                        hT[kt][:, 0:SB],
                        start=(kt == 0), stop=(kt == DT - 1))
                nc.scalar.activation(qT[d][:], ps[:], AF.Identity,
                                     bias=bq_col[:, d:d + 1])

        with (
            tc.tile_pool(name="wvp", bufs=1) as wvp,
            tc.tile_pool(name="bvp", bufs=1) as bvp,
            tc.tile_pool(name="vtp", bufs=3) as vtp,
            tc.tile_pool(name="psD", bufs=4, space="PSUM") as psD,
            tc.tile_pool(name="psDb", bufs=1, space="PSUM") as psDb,
        ):
            bv_row = wvp.tile([1, D], f32r)
            nc.sync.dma_start(bv_row[:], bv[None, :])
            wv_t = []
            for kt in range(DT):
                w = wvp.tile([P, D], f32r, tag=f"wv{kt}", name=f"wv{kt}")
                nc.sync.dma_start(w[:], wv[kt * P:(kt + 1) * P, :])
                wv_t.append(w)
            psb = psDb.tile([P, D], f32, tag="bb", name="bb")
            for ch in range(2):
                nc.tensor.matmul(psb[:, ch * 512:(ch + 1) * 512], ones_r[:],
                                 bv_row[:, ch * 512:(ch + 1) * 512],
                                 start=True, stop=True)
            bv_bc = bvp.tile([P, D], f32, tag="bvbc", name="bvbc")
            nc.scalar.copy(bv_bc[:], psb[:])
            for tt in range(ST):
                for ch in range(2):
                    ps = psD.tile([P, 512], f32, tag="mm", name="mm")
                    for kt in range(DT):
                        nc.tensor.matmul(
                            ps[:], hT[kt][:, tt * P:(tt + 1) * P],
                            wv_t[kt][:, ch * 512:(ch + 1) * 512],
                            start=(kt == 0), stop=(kt == DT - 1))
                    vt = vtp.tile([P, 512], f32r, tag="vt", name="vt")
                    nc.vector.tensor_tensor(vt[:], ps[:],
                                            bv_bc[:, ch * 512:(ch + 1) * 512],
                                            op=OP.add)
                    nc.sync.dma_start(
                        v_buf[tt * P:(tt + 1) * P, ch * 512:(ch + 1) * 512],
                        vt[:])

        # ------------- phase E/F: scores, softmax, transpose -------------
        with (
            tc.tile_pool(name="fp", bufs=2) as fpp,
            tc.tile_pool(name="attn", bufs=2) as attnp,
            tc.tile_pool(name="aTt", bufs=3) as aTtp,
            tc.tile_pool(name="psE", bufs=1, space="PSUM") as psE,
            tc.tile_pool(name="psF", bufs=4, space="PSUM") as psF,
        ):
            cw_col = fpp.tile([P, 1], f32, tag="cw", name="cw")
            nc.sync.dma_start(cw_col[:], cwrap[:])
            wmask = fpp.tile([P, S], f32, tag="wmask", name="wmask")
            nc.gpsimd.iota(wmask[:], pattern=[[1, S]], base=0, channel_multiplier=0,
                           allow_small_or_imprecise_dtypes=True)
            nc.vector.tensor_scalar(wmask[:], wmask[:], cw_col[:, :1], NEG,
                                    OP.is_lt, OP.mult)
            for st in range(SBT):
                psc = psE.tile([P, S], f32, tag="sc", name="sc")
                for ch in range(S // 512):
                    for kt in range(DT):
                        nc.tensor.matmul(
                            psc[:, ch * 512:(ch + 1) * 512],
                            qT[kt][:, st * P:(st + 1) * P],
                            kT[kt][:, ch * 512:(ch + 1) * 512],
                            start=(kt == 0), stop=(kt == DT - 1))
                fmask = fpp.tile([P, S], f32, tag="fmask", name="fmask")
                nc.gpsimd.memset(fmask[:], 0.0)
                nc.gpsimd.affine_select(
                    out=fmask[:], in_=fmask[:], compare_op=OP.is_ge,
                    fill=NEG, base=st * P, pattern=[[-1, S]], channel_multiplier=1)
                nc.vector.tensor_tensor(fmask[:], fmask[:], wmask[:], op=OP.max)
                nc.vector.tensor_tensor(psc[:], psc[:], fmask[:], op=OP.add)
                pst = attnp.tile([P, S], f32, tag="pst", name="pst")
                rs = fpp.tile([P, 1], f32, tag="rs", name="rs")
                nc.scalar.activation(pst[:], psc[:], AF.Exp, accum_out=rs[:, :1])
                rc = fpp.tile([P, 1], f32, tag="rc", name="rc")
                nc.vector.reciprocal(rc[:], rs[:])
                nc.vector.tensor_scalar_mul(pst[:], pst[:], rc[:, :1])
                for kt in range(ST):
                    ps = psF.tile([P, P], f32, tag="tp", name="tp")
                    nc.tensor.transpose(ps[:], pst[:, kt * P:(kt + 1) * P], ident[:])
                    at = aTtp.tile([P, P], f32r, tag="at", name="at")
                    nc.scalar.copy(at[:], ps[:])
                    nc.sync.dma_start(
                        aT_buf[kt * P:(kt + 1) * P, st * P:(st + 1) * P], at[:])
        pclose("ktq")
        pclose("hTp")

        # ---------------- phase G: outT accumulation over keys ----------------
        oTp = popen("oT", bufs=1)
        oT = [oTp.tile([P, SB], f32r, tag=f"oT{m}", name=f"oT{m}") for m in range(DT)]
        with (
            tc.tile_pool(name="vlp", bufs=3) as vlp,
            tc.tile_pool(name="psG", bufs=1, space="PSUM") as psG,
        ):
            pso = [psG.tile([P, SB], f32, tag=f"og{m}", name=f"og{m}") for m in range(DT)]
            for kt in range(ST):
                vkt = vlp.tile([P, D], f32r, tag="vk", name="vk")
                nc.sync.dma_start(vkt[:], v_buf[kt * P:(kt + 1) * P, :])
                akt = vlp.tile([P, SB], f32r, tag="ak", name="ak")
                nc.sync.dma_start(akt[:], aT_buf[kt * P:(kt + 1) * P, :])
                for m in range(DT):
                    nc.tensor.matmul(pso[m][:], vkt[:, m * P:(m + 1) * P], akt[:],
                                     start=(kt == 0), stop=(kt == ST - 1))
            for m in range(DT):
                nc.scalar.copy(oT[m][:], pso[m][:])

        # ---------------- phase H: logits = oT.T @ wo + bo ----------------
        with (
            tc.tile_pool(name="wop", bufs=3) as wop,
            tc.tile_pool(name="lgp", bufs=4) as lgp,
            tc.tile_pool(name="bop", bufs=2) as bop,
            tc.tile_pool(name="psH", bufs=4, space="PSUM") as psH,
            tc.tile_pool(name="psHb", bufs=2, space="PSUM") as psHb,
        ):
            for ch in range(NCH):
                lo = ch * NW
                wo_t = wop.tile([P, DT, NW], f32r, tag="wo", name="wo")
                nc.sync.dma_start(wo_t[:], wo_r[:, :, lo:lo + NW])
                bo_t = bop.tile([1, NW], f32r, tag="bo", name="bo")
                nc.sync.dma_start(bo_t[:], bo[None, lo:lo + NW])
                psb = psHb.tile([P, NW], f32, tag="pb", name="pb")
                nc.tensor.matmul(psb[:], ones_r[:], bo_t[:], start=True, stop=True)
                bo_bc = bop.tile([P, NW], f32, tag="bobc", name="bobc")
                nc.scalar.copy(bo_bc[:], psb[:])
                for m in range(SBT):
                    ps = psH.tile([P, NW], f32, tag="ph", name="ph")
                    for kt in range(DT):
                        nc.tensor.matmul(ps[:], oT[kt][:, m * P:(m + 1) * P],
                                         wo_t[:, kt], start=(kt == 0),
                                         stop=(kt == DT - 1))
                    lg = lgp.tile([P, NW], f32, tag="lg", name="lg")
                    nc.vector.tensor_tensor(lg[:], ps[:], bo_bc[:], op=OP.add)
                    nc.sync.dma_start(logits[m * P:(m + 1) * P, lo:lo + NW], lg[:])
        pclose("oT")
        pclose("misc")


_NC_CACHE = {}


def _get_program():
    if "nc" not in _NC_CACHE:
        nc = bacc.Bacc(None, target_bir_lowering=False, debug=True)
        _emit(nc)
        nc.finalize()
        _NC_CACHE["nc"] = nc
    return _NC_CACHE["nc"]


def kernel(x, tok_emb, pos_emb, wq, bq, wk, bk, wv, bv, wo, bo):
    res, out = run_sharded(x, tok_emb, pos_emb, wq, bq, wk, bk, wv, bv, wo, bo)
    return out


def run_sharded(x, tok_emb, pos_emb, wq, bq, wk, bk, wv, bv, wo, bo, **runkw):
    nc = _get_program()

    x = np.asarray(x, dtype=np.int32)
    tok_emb = np.ascontiguousarray(np.asarray(tok_emb, dtype=np.float32))
    pos_emb = np.asarray(pos_emb, dtype=np.float32)
    wq = np.ascontiguousarray(np.asarray(wq, dtype=np.float32))
    wk = np.ascontiguousarray(np.asarray(wk, dtype=np.float32))
    wv = np.ascontiguousarray(np.asarray(wv, dtype=np.float32))
    wo = np.ascontiguousarray(np.asarray(wo, dtype=np.float32))
    bq = np.asarray(bq, dtype=np.float32)
    bk = np.asarray(bk, dtype=np.float32)
    bv = np.asarray(bv, dtype=np.float32)
    bo = np.asarray(bo, dtype=np.float32)

    B = x.shape[0]
    in_maps = []
    for b in range(B):
        for j in range(4):
            off = j * SB
            in_maps.append({
                "x": np.ascontiguousarray(np.roll(x[b], -off)),
                "pos_t": np.ascontiguousarray(np.roll(pos_emb, -off, axis=0).T),
                "cwrap": np.full((P, 1), float(S - off), np.float32),
                "tok": tok_emb,
                "wq": wq, "wk": wk, "wv": wv,
                "bq": bq, "bk": bk, "bv": bv,
                "wo": wo, "bo": bo,
            })

    res = run_bass_kernel_spmd(nc, in_maps, core_ids=list(range(8)), **runkw)

    out = np.empty((B, S, V), dtype=np.float32)
    for b in range(B):
        for j in range(4):
            out[b, j * SB:(j + 1) * SB, :] = res.results[b * 4 + j]["logits"]
    return res, out

